# revision 28
# baseline (speedup 1.0000x reference)
"""CapsuleLayer Bass/Tile kernel for TRN2 (one NeuronCore; replicated SPMD x8).

Per core: xin [Bc, 2048] f32, kpad [2048, PADN] f32 (kernel cols 0:160,
col 160:176 = 0.1*sum of capsule blocks, rest zero), ident [128, 128].
Output yout [Bc, 16] f32.

Pipeline per 128-sample b-tile:
  DMA xin tile -> SBUF (natural layout)
  PE transpose 16x [128,128] -> PSUM (4 per bank, one accum group per bank)
  DVE/ACT copy PSUM -> SBUF (transposed tiles = matmul lhsT)
  PE matmul (data-as-weights) x16 accumulating kpad-streams -> PSUM hat [128, PADN]
  copy hat[:, :176] -> wide SBUF staging
Routing per group of G b-tiles on wide [128, G*160] layout (DVE/ACT/GPSIMD).
"""

from dataclasses import dataclass

import numpy as np

import concourse.bacc as bacc
import concourse.tile as tile
from concourse import mybir

NCAP = 10
DCAP = 16
EPS = 1e-7
D = 2048
NCOL = NCAP * DCAP  # 160
S1COL = NCOL + DCAP  # 176


@dataclass
class Cfg:
    n_btiles: int = 16          # 128-sample tiles per core
    group: int = 8              # b-tiles per routing group
    pad_n: int = 256            # padded kernel columns streamed per matmul
    data_dt: str = "float32r"   # SBUF dtype for inputs/kpad (matmul operands)
    mm_dt: str = "float32r"     # dtype of main-matmul operands (tt, kpad)
    nat_dt: str = ""            # if "float16": cast x f32->f16 in the DMA
    route_v2: bool = False      # normalize-first fp16 routing
    route_wsum_pool: bool = True  # route_v2: wsum big muls on GPSIMD
    route_spread: bool = False  # route_v2: spread small ops over ACT/Pool
    route_iters: int = 3        # routing iterations (<3 = wrong output, timing only)
    ident_dt: str = "float32r"  # identity dtype (transpose streaming operand)
    copy_split: int = 3         # of 4 stage copies per b-tile, how many on ACT
    n_cores: int = 8
    reps: int = 1               # repeat whole pipeline (for slope timing)
    dma_btiles: int = 1         # b-tiles per input DMA (1/2/4/8)
    ablate: str = "full"        # full | noroute | nomm | dmaonly
    loop_reps: int = 0          # >0: wrap body in a hardware For_i loop
    hat_on_act: bool = True     # hat/s1 PSUM->SBUF copies on ACT (else DVE)
    group_sizes: str = ""       # e.g. "8,5,3"; overrides group when set
    tt_dve_last: int = 0        # how many trailing groups put big TTs on DVE
    nat_bufs: int = 3
    tt_bufs: int = 8
    pstage_bufs: int = 3
    phat_bufs: int = 2

    @property
    def bc(self):
        return self.n_btiles * 128


def make_kpad(kernel: np.ndarray, pad_n: int,
              mm_dt: str = "float32r") -> np.ndarray:
    """[2048, 160] f32 -> [2048, pad_n] with col 160:176 = 0.1 * sum over capsules."""
    d, ncol = kernel.shape
    assert (d, ncol) == (D, NCOL)
    kpad = np.zeros((d, pad_n), dtype=np.float32)
    kpad[:, :NCOL] = kernel
    kpad[:, NCOL:S1COL] = 0.1 * kernel.reshape(d, NCAP, DCAP).sum(axis=1)
    if mm_dt == "bfloat16":
        import ml_dtypes
        kpad = kpad.astype(ml_dtypes.bfloat16)
    elif mm_dt == "float16":
        kpad = kpad.astype(np.float16)
    return kpad


def make_inputs(cfg, x: np.ndarray, kern: np.ndarray) -> list:
    """Per-core input maps for run_bass_kernel_spmd / SpmdRunner."""
    np_idt = {"float32r": np.float32, "float32": np.float32,
              "float16": np.float16}[cfg.ident_dt]
    kpad = make_kpad(np.asarray(kern, dtype=np.float32), cfg.pad_n, cfg.mm_dt)
    # pre-swizzle to the SBUF layout: kp[p, j*PADN+c] = kpad[j*128+p, c]
    kpad = np.ascontiguousarray(
        kpad.reshape(16, 128, cfg.pad_n).transpose(1, 0, 2)
        .reshape(128, 16 * cfg.pad_n))
    ident = np.eye(128, dtype=np_idt)
    seed = np.zeros((128, DCAP), dtype=np.float32)
    bc = cfg.bc
    return [
        {"xin": x[i * bc:(i + 1) * bc], "kpad": kpad, "ident": ident,
         "seed": seed}
        for i in range(cfg.n_cores)
    ]


def build(cfg: Cfg):
    nc = bacc.Bacc("TRN2", target_bir_lowering=False, debug=False,
                   num_devices=cfg.n_cores)
    ddt = getattr(mybir.dt, cfg.data_dt)
    idt = getattr(mybir.dt, cfg.ident_dt)
    mdt = getattr(mybir.dt, cfg.mm_dt)
    ndt = getattr(mybir.dt, cfg.nat_dt) if cfg.nat_dt else ddt
    f32 = mybir.dt.float32
    f16 = mybir.dt.float16
    # transpose matmul requires ident/nat/psum-stage dtypes to agree
    assert idt == ndt, (cfg.ident_dt, cfg.nat_dt, cfg.data_dt)
    hdt = f16 if cfg.route_v2 else f32

    NB = cfg.n_btiles
    PADN = cfg.pad_n
    if cfg.group_sizes:
        sizes = [int(s) for s in cfg.group_sizes.split(",")]
    else:
        assert NB % cfg.group == 0
        sizes = [cfg.group] * (NB // cfg.group)
    assert sum(sizes) == NB

    eps_t = nc.alloc_sbuf_tensor("const-eps", [128, 1], f32)
    nc.gpsimd.memset(eps_t.ap(), EPS)
    nc.const_aps.aps[(f32, EPS)] = eps_t.ap()
    nc.all_engine_barrier()

    xin = nc.dram_tensor("xin", [cfg.bc, D], ddt, kind="ExternalInput")
    kpad = nc.dram_tensor("kpad", [128, 16 * PADN], mdt, kind="ExternalInput")
    ident = nc.dram_tensor("ident", [128, 128], idt, kind="ExternalInput")
    # tiny input consumed by a scratch DMA: lets the bench chain iterations
    # device-side (seed <- slice of yout) to time the NEFF without host RTT
    seed = nc.dram_tensor("seed", [128, DCAP], f32, kind="ExternalInput")
    yout = nc.dram_tensor("yout", [cfg.bc, DCAP], f32, kind="ExternalOutput")

    with tile.TileContext(nc) as tc:
        with (
            tc.tile_pool(name="const", bufs=1) as constp,
            tc.tile_pool(name="nat", bufs=cfg.nat_bufs) as natp,
            tc.tile_pool(name="tT", bufs=cfg.tt_bufs) as tTp,
            tc.tile_pool(name="pstage", bufs=cfg.pstage_bufs, space="PSUM") as pstagep,
            tc.tile_pool(name="phat", bufs=cfg.phat_bufs, space="PSUM") as phatp,
            tc.tile_pool(name="hatw", bufs=2) as hatwp,
            tc.tile_pool(name="rt", bufs=2) as rtp,
            tc.tile_pool(name="sm", bufs=2) as smp,
            tc.tile_pool(name="outs", bufs=2) as outsp,
        ):
            # constants (kpad is loaded after the first xin chunk is queued so
            # the PE's first transposes aren't stuck behind the 2MB const DMA)
            id_t = constp.tile([128, 128], idt, tag="ident")
            nc.sync.dma_start(id_t[:], ident[:, :])
            seed_t = constp.tile([128, DCAP], f32, tag="seed")
            nc.sync.dma_start(seed_t[:], seed[:, :])
            kp_t = constp.tile([128, 16 * PADN], mdt, tag="kpad")

            def load_kpad():
                # host pre-swizzled: kpad[p, j*PADN+c] = K[j*128+p, c]
                nc.sync.dma_start(kp_t[:], kpad[:, :])

            xv = xin[:, :].rearrange("(t p) d -> t p d", p=128)

            # sink for ablation modes: tiny reduces keep DMAs/compute live
            sink = constp.tile([128, 16], f32, tag="sink")

            DB = cfg.dma_btiles
            nat_slices = {}  # i -> (tile, col offset)

            kpad_loaded = [False]

            # dtype cast in the DMA needs the SWDGE (gpsimd) path
            xdma = nc.gpsimd if ndt != ddt else nc.sync

            def load_chunk(i0):
                nat = natp.tile([128, DB * D], ndt, tag="nat")
                if DB == 1:
                    xdma.dma_start(nat[:], xv[i0])
                else:
                    xdma.dma_start(
                        nat[:].rearrange("p (t d) -> p t d", t=DB),
                        xin[:, :].rearrange("(c t p) d -> c p t d",
                                            t=DB, p=128)[i0 // DB],
                    )
                for t in range(DB):
                    nat_slices[i0 + t] = (nat, t * D)
                if not kpad_loaded[0]:
                    kpad_loaded[0] = True
                    load_kpad()

            def run_group(i0, G, gi):
                yv = yout[i0 * 128:(i0 + G) * 128, :].rearrange(
                    "(g p) d -> p g d", p=128)
                hatw = hatwp.tile([128, G * NCOL], hdt, tag="hatw")
                s1w = hatwp.tile([128, G * DCAP], hdt, tag="s1w")
                for g in range(G):
                    i = i0 + g
                    if i % DB == 0:
                        load_chunk(i)
                    nat, off = nat_slices.pop(i)
                    if cfg.ablate == "dmaonly":
                        nc.vector.tensor_reduce(
                            sink[:, :1], nat[:, off:off + 16],
                            axis=mybir.AxisListType.X, op=mybir.AluOpType.add)
                        continue
                    tts = []
                    for c in range(4):
                        ps = pstagep.tile([128, 512], ndt, tag="pstage")
                        for jj in range(4):
                            j = c * 4 + jj
                            nc.tensor.matmul(
                                ps[:, jj * 128:(jj + 1) * 128],
                                nat[:, off + j * 128:off + (j + 1) * 128],
                                id_t[:],
                                is_transpose=True,
                                start=(jj == 0),
                                stop=(jj == 3),
                            )
                        tt = tTp.tile([128, 512], mdt, tag="tT")
                        if c < cfg.copy_split:
                            nc.scalar.copy(tt[:], ps[:])
                        else:
                            nc.vector.tensor_copy(tt[:], ps[:])
                        tts.append(tt)
                    if cfg.ablate == "nomm":
                        for tt in tts:
                            nc.vector.tensor_reduce(
                                sink[:, :1], tt[:, :16],
                                axis=mybir.AxisListType.X, op=mybir.AluOpType.add)
                        continue
                    ph = phatp.tile([128, PADN], f32, tag="phat")
                    for j in range(16):
                        c, jj = divmod(j, 4)
                        nc.tensor.matmul(
                            ph[:],
                            tts[c][:, jj * 128:(jj + 1) * 128],
                            kp_t[:, j * PADN:(j + 1) * PADN],
                            start=(j == 0),
                            stop=(j == 15),
                        )
                    if cfg.hat_on_act:
                        nc.scalar.copy(hatw[:, g * NCOL:(g + 1) * NCOL],
                                       ph[:, :NCOL])
                        nc.scalar.copy(s1w[:, g * DCAP:(g + 1) * DCAP],
                                       ph[:, NCOL:S1COL])
                    else:
                        nc.vector.tensor_copy(hatw[:, g * NCOL:(g + 1) * NCOL],
                                              ph[:, :NCOL])
                        nc.vector.tensor_copy(s1w[:, g * DCAP:(g + 1) * DCAP],
                                              ph[:, NCOL:S1COL])
                if cfg.ablate in ("dmaonly", "nomm"):
                    nc.sync.dma_start(yv[:, :1, :], sink[:].unsqueeze(1))
                    return
                if cfg.ablate == "noroute":
                    nc.sync.dma_start(
                        yv,
                        hatw[:].rearrange("p (g q) -> p g q", g=G)[:, :, :DCAP])
                    return

                if cfg.route_v2:
                    # ---- normalize-first fp16 routing ----
                    Hgnd = hatw[:].rearrange("p (g n d) -> p g n d",
                                             g=G, n=NCAP)
                    gv2 = lambda ap: ap.rearrange("p (g d) -> p g d", g=G)
                    nv2 = lambda ap: ap.rearrange("p (g n) -> p g n", g=G)
                    # engine assignment for the small ops
                    if cfg.route_spread:
                        e_sq = e_c = e_misc = e_t = nc.gpsimd
                        e_den = e_sc = nc.vector  # STT is DVE-only
                    else:
                        e_sq = e_den = e_c = e_misc = nc.vector
                        e_sc = e_t = nc.vector

                    def recip(out, in_):
                        nc.vector.reciprocal(out, in_)

                    def squash_sc(su, tag):
                        """squash scale for an already-normalized s: [128,G]"""
                        sq = smp.tile([128, G * DCAP], f32, tag=f"sq{tag}")
                        e_sq.tensor_mul(sq[:], su, su)
                        m2 = smp.tile([128, G], f32, tag=f"m2{tag}")
                        nc.vector.tensor_reduce(
                            m2[:], gv2(sq[:]),
                            axis=mybir.AxisListType.X, op=mybir.AluOpType.add)
                        sr = smp.tile([128, G], f32, tag=f"sr{tag}")
                        nc.scalar.activation(
                            sr[:], m2[:],
                            mybir.ActivationFunctionType.Sqrt, bias=EPS)
                        den = smp.tile([128, G], f32, tag=f"den{tag}")
                        e_den.scalar_tensor_tensor(
                            den[:], m2[:], 1.0, sr[:],
                            op0=mybir.AluOpType.add, op1=mybir.AluOpType.mult)
                        rec = smp.tile([128, G], f32, tag=f"rec{tag}")
                        recip(rec[:], den[:])
                        sc = smp.tile([128, G], f32, tag=f"sc{tag}")
                        e_sc.tensor_mul(sc[:], m2[:], rec[:])
                        return sc

                    def dots_d(src16, tag):
                        """r[g,n] = sum_d H[g,n,d]*src[g,d], src fp16"""
                        tmp = rtp.tile([128, G * NCOL], f16, tag=f"dt{tag}")
                        bc = gv2(src16).unsqueeze(2).broadcast_to(
                            (128, G, NCAP, DCAP))
                        nc.vector.tensor_mul(
                            tmp[:].rearrange("p (g n d) -> p g n d",
                                             g=G, n=NCAP), Hgnd, bc)
                        out = rtp.tile([128, G * NCAP], f32, tag=f"dr{tag}")
                        nc.vector.tensor_reduce(
                            out[:],
                            tmp[:].rearrange("p (g n d) -> p g n d",
                                             g=G, n=NCAP),
                            axis=mybir.AxisListType.X, op=mybir.AluOpType.add)
                        return out

                    def wsum_n(c16, tag):
                        """s[g,d] = sum_n H[g,n,d]*c[g,n], c fp16"""
                        tmp = rtp.tile([128, G * NCOL], f16, tag=f"wt{tag}")
                        bc = nv2(c16).unsqueeze(3).broadcast_to(
                            (128, G, NCAP, DCAP))
                        weng = nc.gpsimd if cfg.route_wsum_pool else nc.vector
                        weng.tensor_mul(
                            tmp[:].rearrange("p (g n d) -> p g n d",
                                             g=G, n=NCAP), Hgnd, bc)
                        out = rtp.tile([128, G * DCAP], f32, tag=f"ws{tag}")
                        nc.vector.tensor_reduce(
                            out[:],
                            tmp[:].rearrange("p (g n d) -> p g d n",
                                             g=G, n=NCAP),
                            axis=mybir.AxisListType.X, op=mybir.AluOpType.add)
                        return out

                    def softmax_c(t, tag):
                        """c = softmax_n(t) cast to fp16"""
                        e = rtp.tile([128, G * NCAP], f32, tag=f"e{tag}")
                        nc.scalar.activation(
                            e[:], t, mybir.ActivationFunctionType.Exp)
                        se = smp.tile([128, G], f32, tag=f"se{tag}")
                        nc.vector.tensor_reduce(
                            se[:], nv2(e[:]),
                            axis=mybir.AxisListType.X, op=mybir.AluOpType.add)
                        ri = smp.tile([128, G], f32, tag=f"ri{tag}")
                        recip(ri[:], se[:])
                        c = rtp.tile([128, G * NCAP], f16, tag=f"c{tag}")
                        e_c.tensor_mul(
                            nv2(c[:]), nv2(e[:]),
                            ri[:].unsqueeze(2).broadcast_to((128, G, NCAP)))
                        return c

                    comb1 = squash_sc(s1w[:], "1")
                    if cfg.route_iters == 1:
                        v3 = outsp.tile([128, G * DCAP], f32, tag="v3")
                        e_misc.tensor_mul(
                            gv2(v3[:]), gv2(s1w[:]),
                            comb1[:].unsqueeze(2).broadcast_to((128, G, DCAP)))
                        nc.sync.dma_start(yv, gv2(v3[:]))
                        return
                    r2 = dots_d(s1w[:], "2")
                    t2 = rtp.tile([128, G * NCAP], f32, tag="t2")
                    e_t.tensor_mul(
                        nv2(t2[:]), nv2(r2[:]),
                        comb1[:].unsqueeze(2).broadcast_to((128, G, NCAP)))
                    c2 = softmax_c(t2[:], "2")
                    s2 = wsum_n(c2[:], "2")
                    comb2 = squash_sc(s2[:], "2")
                    if cfg.route_iters == 2:
                        v3 = outsp.tile([128, G * DCAP], f32, tag="v3")
                        e_misc.tensor_mul(
                            gv2(v3[:]), gv2(s2[:]),
                            comb2[:].unsqueeze(2).broadcast_to((128, G, DCAP)))
                        nc.sync.dma_start(yv, gv2(v3[:]))
                        return
                    s2h = smp.tile([128, G * DCAP], f16, tag="s2h")
                    e_misc.tensor_copy(s2h[:], s2[:])
                    r3 = dots_d(s2h[:], "3")
                    t3 = rtp.tile([128, G * NCAP], f32, tag="t3")
                    e_t.tensor_mul(
                        nv2(t3[:]), nv2(r3[:]),
                        comb2[:].unsqueeze(2).broadcast_to((128, G, NCAP)))
                    e_t.tensor_add(t3[:], t3[:], t2[:])
                    c3 = softmax_c(t3[:], "3")
                    s3 = wsum_n(c3[:], "3")
                    comb3 = squash_sc(s3[:], "3")
                    v3 = outsp.tile([128, G * DCAP], f32, tag="v3")
                    e_misc.tensor_mul(
                        gv2(v3[:]), gv2(s3[:]),
                        comb3[:].unsqueeze(2).broadcast_to((128, G, DCAP)))
                    nc.sync.dma_start(yv, gv2(v3[:]))
                    return

                # ---- routing on [128, G*160] ----
                tt_eng = (nc.vector
                          if gi >= len(sizes) - int(cfg.tt_dve_last)
                          else nc.gpsimd)
                H = hatw[:]
                Hgnd = H.rearrange("p (g n d) -> p g n d", g=G, n=NCAP)

                def squash_comb(su, r, tag):
                    """combined scale c s.t. v = c * su, where s = su * r
                    (r None -> s = su). Returns [128, G] AP."""
                    sq = smp.tile([128, G * DCAP], f32, tag=f"sq{tag}")
                    nc.vector.tensor_mul(sq[:], su, su)
                    m2 = smp.tile([128, G], f32, tag=f"m2{tag}")
                    nc.vector.tensor_reduce(
                        m2[:], sq[:].rearrange("p (g d) -> p g d", g=G),
                        axis=mybir.AxisListType.X, op=mybir.AluOpType.add)
                    if r is not None:
                        rr = smp.tile([128, G], f32, tag=f"rr{tag}")
                        nc.vector.tensor_mul(rr[:], r, r)
                        n2 = smp.tile([128, G], f32, tag=f"n2{tag}")
                        nc.vector.tensor_mul(n2[:], m2[:], rr[:])
                    else:
                        n2 = m2
                    sr = smp.tile([128, G], f32, tag=f"sr{tag}")
                    nc.scalar.activation(sr[:], n2[:],
                                         mybir.ActivationFunctionType.Sqrt,
                                         bias=EPS)
                    den = smp.tile([128, G], f32, tag=f"den{tag}")
                    nc.vector.scalar_tensor_tensor(
                        den[:], n2[:], 1.0, sr[:],
                        op0=mybir.AluOpType.add, op1=mybir.AluOpType.mult)
                    rec = smp.tile([128, G], f32, tag=f"rec{tag}")
                    nc.vector.reciprocal(rec[:], den[:])
                    sc = smp.tile([128, G], f32, tag=f"sc{tag}")
                    nc.vector.tensor_mul(sc[:], n2[:], rec[:])
                    if r is not None:
                        comb = smp.tile([128, G], f32, tag=f"comb{tag}")
                        nc.vector.tensor_mul(comb[:], sc[:], r)
                        return comb
                    return sc

                def dots_d(src_gd, tag):
                    """r[g,n] = sum_d H[g,n,d] * src[g,d] -> [128, G*NCAP]"""
                    tmp = rtp.tile([128, G * NCOL], f32, tag=f"dt{tag}")
                    bc = src_gd.unsqueeze(2).broadcast_to((128, G, NCAP, DCAP))
                    tt_eng.tensor_mul(
                        tmp[:].rearrange("p (g n d) -> p g n d", g=G, n=NCAP),
                        Hgnd, bc)
                    out = rtp.tile([128, G * NCAP], f32, tag=f"dr{tag}")
                    nc.vector.tensor_reduce(
                        out[:], tmp[:].rearrange("p (g n d) -> p g n d", g=G, n=NCAP),
                        axis=mybir.AxisListType.X, op=mybir.AluOpType.add)
                    return out

                def wsum_n(e_gn, tag):
                    """su[g,d] = sum_n H[g,n,d] * e[g,n] -> [128, G*DCAP]"""
                    tmp = rtp.tile([128, G * NCOL], f32, tag=f"wt{tag}")
                    bc = e_gn.unsqueeze(3).broadcast_to((128, G, NCAP, DCAP))
                    tt_eng.tensor_mul(
                        tmp[:].rearrange("p (g n d) -> p g n d", g=G, n=NCAP),
                        Hgnd, bc)
                    out = rtp.tile([128, G * DCAP], f32, tag=f"ws{tag}")
                    nc.vector.tensor_reduce(
                        out[:], tmp[:].rearrange("p (g n d) -> p g d n", g=G, n=NCAP),
                        axis=mybir.AxisListType.X, op=mybir.AluOpType.add)
                    return out

                def softmax_recip(t_gn, tag):
                    """e = exp(t) [128, G*NCAP]; r = 1/sum_n e [128, G]"""
                    e = rtp.tile([128, G * NCAP], f32, tag=f"e{tag}")
                    nc.scalar.activation(e[:], t_gn,
                                         mybir.ActivationFunctionType.Exp)
                    se = smp.tile([128, G], f32, tag=f"se{tag}")
                    nc.vector.tensor_reduce(
                        se[:], e[:].rearrange("p (g n) -> p g n", g=G),
                        axis=mybir.AxisListType.X, op=mybir.AluOpType.add)
                    ri = smp.tile([128, G], f32, tag=f"ri{tag}")
                    nc.vector.reciprocal(ri[:], se[:])
                    return e, ri

                gv = lambda ap: ap.rearrange("p (g d) -> p g d", g=G)
                nv = lambda ap: ap.rearrange("p (g n) -> p g n", g=G)

                # iter 1: s1 (pre-scaled mean) came from the matmul
                comb1 = squash_comb(s1w[:], None, "1")  # v1 = comb1*s1
                r2 = dots_d(gv(s1w[:]), "2")            # u.s1
                t2 = rtp.tile([128, G * NCAP], f32, tag="t2")
                nc.vector.tensor_mul(
                    nv(t2[:]), nv(r2[:]),
                    comb1[:].rearrange("p g -> p g").unsqueeze(2)
                    .broadcast_to((128, G, NCAP)))

                # iter 2
                e2, r2i = softmax_recip(t2[:], "2")
                s2u = wsum_n(nv(e2[:]), "2")
                comb2 = squash_comb(s2u[:], r2i[:], "2")  # v2 = comb2*s2u
                r3 = dots_d(gv(s2u[:]), "3")              # u.s2u
                t3 = rtp.tile([128, G * NCAP], f32, tag="t3")
                nc.vector.tensor_mul(
                    nv(t3[:]), nv(r3[:]),
                    comb2[:].unsqueeze(2).broadcast_to((128, G, NCAP)))
                nc.vector.tensor_add(t3[:], t3[:], t2[:])

                # iter 3
                e3, r3i = softmax_recip(t3[:], "3")
                s3u = wsum_n(nv(e3[:]), "3")
                comb3 = squash_comb(s3u[:], r3i[:], "3")
                v3 = outsp.tile([128, G * DCAP], f32, tag="v3")
                nc.vector.tensor_mul(
                    gv(v3[:]), gv(s3u[:]),
                    comb3[:].unsqueeze(2).broadcast_to((128, G, DCAP)))
                nc.sync.dma_start(
                    yv,
                    v3[:].rearrange("p (g d) -> p g d", g=G))

            def run_all():
                i0 = 0
                for gi, G in enumerate(sizes):
                    run_group(i0, G, gi)
                    i0 += G

            if cfg.loop_reps > 0:
                with tc.For_i(0, cfg.loop_reps, 1,
                              hint_engines=(mybir.EngineType.PE,)):
                    run_all()
            else:
                for _rep in range(cfg.reps):
                    run_all()

    nc.compile()
    return nc


# ---------------- numpy reference (per-core) ----------------

def ref_numpy(x: np.ndarray, kernel: np.ndarray) -> np.ndarray:
    b = x.shape[0]
    hat = (x @ kernel).reshape(b, NCAP, DCAP)
    logits = np.zeros((b, NCAP, 1), dtype=x.dtype)
    out = None
    for _ in range(3):
        ex = np.exp(logits - logits.max(axis=1, keepdims=True))
        c = ex / ex.sum(axis=1, keepdims=True)
        s = (c * hat).sum(axis=1, keepdims=True)
        s2 = np.square(s).sum(axis=-1, keepdims=True)
        out = s2 / (1.0 + s2) / np.sqrt(s2 + EPS) * s
        logits = logits + np.einsum("bnd,bd->bn", hat, out[:, 0, :])[:, :, None]
    return out[:, 0, :]


# ---------------- public entry point ----------------

_CACHE = {}

BEST = Cfg(n_btiles=16, group_sizes="10,4,2", tt_dve_last=2,
           nat_bufs=6, tt_bufs=16, pstage_bufs=4, phat_bufs=3,
           nat_dt="float16", ident_dt="float16", mm_dt="float16",
           pad_n=176, route_v2=True, route_spread=True, copy_split=2)


def kernel(inputs: np.ndarray, kernel: np.ndarray) -> np.ndarray:
    """CapsuleLayer forward: inputs [16384, 2048] f32, kernel [2048, 160] f32
    -> [16384, 16] f32. Runs SPMD across 8 NeuronCores (batch split 8 ways)."""
    from concourse.bass_utils import run_bass_kernel_spmd

    cfg = BEST
    assert inputs.shape == (cfg.bc * cfg.n_cores, D)
    assert kernel.shape == (D, NCOL)
    if "nc" not in _CACHE:
        _CACHE["nc"] = build(cfg)
    nc = _CACHE["nc"]

    x = np.ascontiguousarray(inputs, dtype=np.float32)
    in_maps = make_inputs(cfg, x, kernel)
    res = run_bass_kernel_spmd(nc, in_maps, list(range(cfg.n_cores)))
    return np.concatenate(
        [res.results[i]["yout"] for i in range(cfg.n_cores)], axis=0)

